# revision 40
# baseline (speedup 1.0000x reference)
"""Radial power-spectrum (GroupStat.get_spectrum) Trainium2 kernel.

Math:  out[b,c,r] = sum_{p: idx[p]==r} x[b,c,p]^2 * w[p] / (cnt[r]+eps)

Strategy (8 NeuronCores, data-parallel over batch B=128 -> 16 per core):
  * per core n = 16*8 = 128 rows (b_local, c) -> exactly the 128 SBUF
    partitions; pixels p = 256*129 = 33024 = 258 chunks of 128.
  * fold w[p]/(cnt[idx[p]]+eps) into a single per-pixel scalar wt[p] (host).
  * host also ships x pre-transposed per shard as xt[p, chunk, n] so the
    device never transposes: each chunk slab is directly the matmul lhsT.
  * device pipeline (DMA stream is the serialized bottleneck; everything
    else hides under it):
      - DVE: pregenerate all 258 weighted one-hot tiles [128p, 130r]
        = (iota == idx[p]) * wt[p]  (tensor_scalar is_equal+mult), each in
        its own buffer so nothing ever waits on buffer recycling.
      - DMA: stream xt in big tiles [128p, J, 128n] fp32, back-to-back and
        gap-free; J tapers down to 1 at the end so the last tile's
        square+matmul chain after the final byte is minimal.
      - ScalarE/DVE: square, scale 1024, cast -> fp16 (keeps tiny x^2 out
        of fp16 subnormals; undone in the final copy).
      - PE: psum[128n, 130r] += slab.T @ onehot  (258 accumulating matmuls).
  * psum -> SBUF -> DRAM [128, 129] per core; host stacks to [128,8,129].
"""

import numpy as np

from contextlib import ExitStack

from concourse import bacc, mybir
import concourse.tile as tile
from concourse.bass_utils import run_bass_kernel_spmd

B, C, S, XDIM = 128, 8, 256, 129
MAX_R = XDIM  # 129 shells
EPS = 1e-5
NCORES = 8
BLOC = B // NCORES          # 16 batches per core
NROW = BLOC * C             # 128 rows per core -> partition dim
NPIX = S * XDIM             # 33024 pixels
PCHUNK = 128
NCHUNK = NPIX // PCHUNK     # 258 (exact)
RPAD = 130                  # even free dim for DVE 4x mode; col 129 unused

# chunks per load tile; tapered so the tail tiles' square+matmul work is
# tiny once the DMA stream (the serialized bottleneck) finishes
TILES = [32, 32, 32, 32, 32, 32, 16, 16, 12, 12, 4, 3, 2, 1]
# explicit tail square engine map (True = Act): keeps the proven
# alternation with the last square on Act regardless of tile count
ACT_SQ = {8: True, 9: False, 10: False, 11: True, 12: False, 13: True}
assert sum(TILES) == NCHUNK

F32 = mybir.dt.float32
F16 = mybir.dt.float16

_CACHE: dict = {}


def _build_program():
    nc = bacc.Bacc("TRN2", target_bir_lowering=False, debug=False,
                   num_devices=NCORES)

    x_d = nc.dram_tensor("xt", [PCHUNK, NCHUNK, NROW], F32,
                         kind="ExternalInput").ap()
    # idx / wt chunk-transposed and packed: [128, 2, NCHUNK];
    # [:, 0, c] = shell index for chunk c, [:, 1, c] = folded weight.
    # shipped f16 (idx is small ints, wt lands in the f16 one-hot anyway)
    # and upconverted on-device: is_equal needs an f32 scalar operand
    iw_d = nc.dram_tensor("iw", [PCHUNK, 2, NCHUNK], F16,
                          kind="ExternalInput").ap()
    out_d = nc.dram_tensor("out", [NROW, MAX_R], F32,
                           kind="ExternalOutput").ap()

    with tile.TileContext(nc) as tc, ExitStack() as ctx:
        const_pool = ctx.enter_context(tc.tile_pool(name="const", bufs=1))
        xin_pool = ctx.enter_context(tc.tile_pool(name="xin", bufs=3))
        x2_pool = ctx.enter_context(tc.tile_pool(name="x2", bufs=3))
        # one buffer per tail tile: a back-pressured tail DMA exposes the
        # ~1.9us issue->transfer pipeline, so tail buffers never recycle
        xin_s_pool = ctx.enter_context(tc.tile_pool(name="xin_s", bufs=5))
        x2_s_pool = ctx.enter_context(tc.tile_pool(name="x2_s", bufs=5))
        oh_pool = ctx.enter_context(tc.tile_pool(name="oh", bufs=NCHUNK))
        acc_pool = ctx.enter_context(
            tc.tile_pool(name="acc", bufs=1, space="PSUM"))

        # consts go over the Activation queue so SP's x-tile stream is
        # never interrupted; iota is generated on the idle GPSIMD engine
        iw_h = const_pool.tile([PCHUNK, 2, NCHUNK], F16)
        nc.scalar.dma_start(iw_h[:], iw_d[:])
        iw_t = const_pool.tile([PCHUNK, 2, NCHUNK], F32)
        nc.vector.tensor_copy(iw_t[:], iw_h[:])
        idx_t = iw_t[:, 0, :]
        wt_t = iw_t[:, 1, :]
        iota_t = const_pool.tile([PCHUNK, RPAD], F16)
        nc.gpsimd.iota(iota_t[:], [[1, RPAD]], channel_multiplier=0,
                       allow_small_or_imprecise_dtypes=True)

        # all 258 one-hot tiles upfront on DVE; runs under the x DMA stream
        ohs = []
        for c in range(NCHUNK):
            oh = oh_pool.tile([PCHUNK, RPAD], F16, tag="oh")
            nc.vector.tensor_scalar(
                oh[:], iota_t[:],
                scalar1=idx_t[:, c:c + 1], scalar2=wt_t[:, c:c + 1],
                op0=mybir.AluOpType.is_equal,
                op1=mybir.AluOpType.mult)
            ohs.append(oh)

        acc = acc_pool.tile([NROW, RPAD], F32)
        jmax = max(TILES)
        jsmall = 12

        c = 0
        for ti, nj in enumerate(TILES):
            t0 = c
            if nj > jsmall:
                xin = xin_pool.tile([PCHUNK, jmax, NROW], F32, tag="xin")
                x2 = x2_pool.tile([PCHUNK, jmax, NROW], F16, tag="x2")
            else:
                xin = xin_s_pool.tile([PCHUNK, jsmall, NROW], F32, tag="xs")
                x2 = x2_s_pool.tile([PCHUNK, jsmall, NROW], F16, tag="x2s")
            nc.sync.dma_start(xin[:, :nj, :], x_d[:, t0:t0 + nj, :])
            # values are 1024*x^2: keeps tiny x^2 out of fp16 subnormals
            # (undone by the 1/1024 in the final copy). tail tiles
            # alternate Act/DVE so the last squares aren't serialized
            # behind one engine once the DMA stream ends
            if nj > 8 and ti not in ACT_SQ or ACT_SQ.get(ti, True):
                nc.scalar.activation(x2[:, :nj, :], xin[:, :nj, :],
                                     mybir.ActivationFunctionType.Square,
                                     scale=32.0)
            else:
                nc.vector.scalar_tensor_tensor(
                    x2[:, :nj, :], xin[:, :nj, :], 1024.0, xin[:, :nj, :],
                    op0=mybir.AluOpType.mult, op1=mybir.AluOpType.mult)
            for j in range(nj):
                nc.tensor.matmul(acc[:], lhsT=x2[:, j, :], rhs=ohs[c][:],
                                 start=(c == 0), stop=(c == NCHUNK - 1))
                c += 1
        assert c == NCHUNK

        res = const_pool.tile([NROW, MAX_R], F32)
        nc.vector.tensor_scalar_mul(res[:], acc[:, :MAX_R], 1.0 / 1024.0)
        nc.sync.dma_start(out_d[:], res[:])

    nc.compile()
    return nc


def _get_program():
    if "nc" not in _CACHE:
        _CACHE["nc"] = _build_program()
    return _CACHE["nc"]


def _host_prep(shell_index: np.ndarray, shells_weight: np.ndarray,
               shells_count: np.ndarray):
    idx_flat = shell_index.reshape(-1).astype(np.int64)
    wt = shells_weight.reshape(-1).astype(np.float64) / (
        shells_count.astype(np.float64)[idx_flat] + EPS)
    # chunk-transpose: A[i, c] = v[c*128 + i]; pack idx+wt into one tensor
    idx_t = idx_flat.reshape(NCHUNK, PCHUNK).T.astype(np.float32)
    wt_t = wt.reshape(NCHUNK, PCHUNK).T.astype(np.float32)
    iw = np.ascontiguousarray(
        np.stack([idx_t, wt_t], axis=1)).astype(np.float16)
    return iw


def kernel(x: np.ndarray, shell_index: np.ndarray,
           shells_weight: np.ndarray, shells_count: np.ndarray,
           _trace: bool = False, **_tr_kwargs) -> np.ndarray:
    x = np.asarray(x)
    shell_index = np.asarray(shell_index)
    shells_weight = np.asarray(shells_weight)
    shells_count = np.asarray(shells_count)
    assert x.shape == (B, C, S, XDIM)
    nc = _get_program()
    iw = _host_prep(shell_index, shells_weight, shells_count)

    x = np.ascontiguousarray(x, dtype=np.float32)
    in_maps = []
    for k in range(NCORES):
        xk = x[k * BLOC:(k + 1) * BLOC].reshape(NROW, NCHUNK, PCHUNK)
        # shard layout [pixel-in-chunk, chunk, row]: each chunk slab is the
        # matmul lhsT, so the device does no transposes at all
        xkt = np.ascontiguousarray(xk.transpose(2, 1, 0))
        in_maps.append({"xt": xkt, "iw": iw})

    res = run_bass_kernel_spmd(nc, in_maps, list(range(NCORES)),
                               trace=_trace, **_tr_kwargs)
    outs = [res.results[k]["out"] for k in range(NCORES)]
    full = np.concatenate(outs, axis=0).reshape(B, C, MAX_R).astype(np.float32)
    if _trace:
        return full, res
    return full


# revision 41
# speedup vs baseline: 1.0075x; 1.0075x over previous
"""Radial power-spectrum (GroupStat.get_spectrum) Trainium2 kernel.

Math:  out[b,c,r] = sum_{p: idx[p]==r} x[b,c,p]^2 * w[p] / (cnt[r]+eps)

Strategy (8 NeuronCores, data-parallel over batch B=128 -> 16 per core):
  * per core n = 16*8 = 128 rows (b_local, c) -> exactly the 128 SBUF
    partitions; pixels p = 256*129 = 33024 = 258 chunks of 128.
  * fold w[p]/(cnt[idx[p]]+eps) into a single per-pixel scalar wt[p] (host).
  * host also ships x pre-transposed per shard as xt[p, chunk, n] so the
    device never transposes: each chunk slab is directly the matmul lhsT.
  * device pipeline (DMA stream is the serialized bottleneck; everything
    else hides under it):
      - DVE: pregenerate all 258 weighted one-hot tiles [128p, 130r]
        = (iota == idx[p]) * wt[p]  (tensor_scalar is_equal+mult), each in
        its own buffer so nothing ever waits on buffer recycling.
      - DMA: stream xt in big tiles [128p, J, 128n] fp32, back-to-back and
        gap-free; J tapers down to 1 at the end so the last tile's
        square+matmul chain after the final byte is minimal.
      - ScalarE/DVE: square, scale 1024, cast -> fp16 (keeps tiny x^2 out
        of fp16 subnormals; undone in the final copy).
      - PE: psum[128n, 130r] += slab.T @ onehot  (258 accumulating matmuls).
  * psum -> SBUF -> DRAM [128, 129] per core; host stacks to [128,8,129].
"""

import numpy as np

from contextlib import ExitStack

from concourse import bacc, mybir
import concourse.tile as tile
from concourse.bass_utils import run_bass_kernel_spmd

B, C, S, XDIM = 128, 8, 256, 129
MAX_R = XDIM  # 129 shells
EPS = 1e-5
NCORES = 8
BLOC = B // NCORES          # 16 batches per core
NROW = BLOC * C             # 128 rows per core -> partition dim
NPIX = S * XDIM             # 33024 pixels
PCHUNK = 128
NCHUNK = NPIX // PCHUNK     # 258 (exact)
RPAD = 130                  # even free dim for DVE 4x mode; col 129 unused

# chunks per load tile; tapered so the tail tiles' square+matmul work is
# tiny once the DMA stream (the serialized bottleneck) finishes
TILES = [32, 32, 32, 32, 32, 32, 16, 16, 8, 8, 8, 4, 3, 2, 1]
assert sum(TILES) == NCHUNK

F32 = mybir.dt.float32
F16 = mybir.dt.float16

_CACHE: dict = {}


def _build_program():
    nc = bacc.Bacc("TRN2", target_bir_lowering=False, debug=False,
                   num_devices=NCORES)

    x_d = nc.dram_tensor("xt", [PCHUNK, NCHUNK, NROW], F32,
                         kind="ExternalInput").ap()
    # idx / wt chunk-transposed and packed: [128, 2, NCHUNK];
    # [:, 0, c] = shell index for chunk c, [:, 1, c] = folded weight.
    # shipped f16 (idx is small ints, wt lands in the f16 one-hot anyway)
    # and upconverted on-device: is_equal needs an f32 scalar operand
    iw_d = nc.dram_tensor("iw", [PCHUNK, 2, NCHUNK], F16,
                          kind="ExternalInput").ap()
    out_d = nc.dram_tensor("out", [NROW, MAX_R], F32,
                           kind="ExternalOutput").ap()

    with tile.TileContext(nc) as tc, ExitStack() as ctx:
        const_pool = ctx.enter_context(tc.tile_pool(name="const", bufs=1))
        xin_pool = ctx.enter_context(tc.tile_pool(name="xin", bufs=3))
        x2_pool = ctx.enter_context(tc.tile_pool(name="x2", bufs=3))
        # one buffer per tail tile: a back-pressured tail DMA exposes the
        # ~1.9us issue->transfer pipeline, so tail buffers never recycle
        xin_s_pool = ctx.enter_context(tc.tile_pool(name="xin_s", bufs=7))
        x2_s_pool = ctx.enter_context(tc.tile_pool(name="x2_s", bufs=7))
        oh_pool = ctx.enter_context(tc.tile_pool(name="oh", bufs=NCHUNK))
        acc_pool = ctx.enter_context(
            tc.tile_pool(name="acc", bufs=1, space="PSUM"))

        # consts go over the Activation queue so SP's x-tile stream is
        # never interrupted; iota is generated on the idle GPSIMD engine
        iw_h = const_pool.tile([PCHUNK, 2, NCHUNK], F16)
        nc.scalar.dma_start(iw_h[:], iw_d[:])
        iw_t = const_pool.tile([PCHUNK, 2, NCHUNK], F32)
        nc.vector.tensor_copy(iw_t[:], iw_h[:])
        idx_t = iw_t[:, 0, :]
        wt_t = iw_t[:, 1, :]
        iota_t = const_pool.tile([PCHUNK, RPAD], F16)
        nc.gpsimd.iota(iota_t[:], [[1, RPAD]], channel_multiplier=0,
                       allow_small_or_imprecise_dtypes=True)

        # all 258 one-hot tiles upfront on DVE; runs under the x DMA stream
        ohs = []
        for c in range(NCHUNK):
            oh = oh_pool.tile([PCHUNK, RPAD], F16, tag="oh")
            nc.vector.tensor_scalar(
                oh[:], iota_t[:],
                scalar1=idx_t[:, c:c + 1], scalar2=wt_t[:, c:c + 1],
                op0=mybir.AluOpType.is_equal,
                op1=mybir.AluOpType.mult)
            ohs.append(oh)

        acc = acc_pool.tile([NROW, RPAD], F32)
        jmax = max(TILES)
        jsmall = 8

        c = 0
        for ti, nj in enumerate(TILES):
            t0 = c
            if nj > jsmall:
                xin = xin_pool.tile([PCHUNK, jmax, NROW], F32, tag="xin")
                x2 = x2_pool.tile([PCHUNK, jmax, NROW], F16, tag="x2")
            else:
                xin = xin_s_pool.tile([PCHUNK, jsmall, NROW], F32, tag="xs")
                x2 = x2_s_pool.tile([PCHUNK, jsmall, NROW], F16, tag="x2s")
            nc.sync.dma_start(xin[:, :nj, :], x_d[:, t0:t0 + nj, :])
            # values are 1024*x^2: keeps tiny x^2 out of fp16 subnormals
            # (undone by the 1/1024 in the final copy). tail tiles
            # alternate Act/DVE so the last squares aren't serialized
            # behind one engine once the DMA stream ends
            if nj > 8 or (ti % 2 == 0):
                nc.scalar.activation(x2[:, :nj, :], xin[:, :nj, :],
                                     mybir.ActivationFunctionType.Square,
                                     scale=32.0)
            else:
                nc.vector.scalar_tensor_tensor(
                    x2[:, :nj, :], xin[:, :nj, :], 1024.0, xin[:, :nj, :],
                    op0=mybir.AluOpType.mult, op1=mybir.AluOpType.mult)
            for j in range(nj):
                nc.tensor.matmul(acc[:], lhsT=x2[:, j, :], rhs=ohs[c][:],
                                 start=(c == 0), stop=(c == NCHUNK - 1))
                c += 1
        assert c == NCHUNK

        res = const_pool.tile([NROW, MAX_R], F32)
        nc.vector.tensor_scalar_mul(res[:], acc[:, :MAX_R], 1.0 / 1024.0)
        nc.sync.dma_start(out_d[:], res[:])

    nc.compile()
    return nc


def _get_program():
    if "nc" not in _CACHE:
        _CACHE["nc"] = _build_program()
    return _CACHE["nc"]


def _host_prep(shell_index: np.ndarray, shells_weight: np.ndarray,
               shells_count: np.ndarray):
    idx_flat = shell_index.reshape(-1).astype(np.int64)
    wt = shells_weight.reshape(-1).astype(np.float64) / (
        shells_count.astype(np.float64)[idx_flat] + EPS)
    # chunk-transpose: A[i, c] = v[c*128 + i]; pack idx+wt into one tensor
    idx_t = idx_flat.reshape(NCHUNK, PCHUNK).T.astype(np.float32)
    wt_t = wt.reshape(NCHUNK, PCHUNK).T.astype(np.float32)
    iw = np.ascontiguousarray(
        np.stack([idx_t, wt_t], axis=1)).astype(np.float16)
    return iw


def kernel(x: np.ndarray, shell_index: np.ndarray,
           shells_weight: np.ndarray, shells_count: np.ndarray,
           _trace: bool = False, **_tr_kwargs) -> np.ndarray:
    x = np.asarray(x)
    shell_index = np.asarray(shell_index)
    shells_weight = np.asarray(shells_weight)
    shells_count = np.asarray(shells_count)
    assert x.shape == (B, C, S, XDIM)
    nc = _get_program()
    iw = _host_prep(shell_index, shells_weight, shells_count)

    x = np.ascontiguousarray(x, dtype=np.float32)
    in_maps = []
    for k in range(NCORES):
        xk = x[k * BLOC:(k + 1) * BLOC].reshape(NROW, NCHUNK, PCHUNK)
        # shard layout [pixel-in-chunk, chunk, row]: each chunk slab is the
        # matmul lhsT, so the device does no transposes at all
        xkt = np.ascontiguousarray(xk.transpose(2, 1, 0))
        in_maps.append({"xt": xkt, "iw": iw})

    res = run_bass_kernel_spmd(nc, in_maps, list(range(NCORES)),
                               trace=_trace, **_tr_kwargs)
    outs = [res.results[k]["out"] for k in range(NCORES)]
    full = np.concatenate(outs, axis=0).reshape(B, C, MAX_R).astype(np.float32)
    if _trace:
        return full, res
    return full
